# revision 1
# baseline (speedup 1.0000x reference)
"""Trainium2 Bass kernel for MetaDynamics potential evaluation.

out[p] = sum_h hgt[h] * exp(-0.5 * sum_d (cen[h,d]-col[p,d])^2 / wdt[h,d]^2)
with H=16384 hills, P=4096 points, D=8 collective variables.

Algorithm: expand the quadratic form into a rank-17 inner product
  e'[h,p] = sum_d (cen*c)[h,d]*col[p,d] - 0.5*sum_d c[h,d]*col[p,d]^2 - 0.5*a[h]
  c = 1/wdt^2, a[h] = sum_d cen^2*c - 2*ln(hgt[h]);   out[p] = sum_h exp(e'[h,p])
so e' = W~ @ F^T is a K=17 matmul (W~=[cen*c, -c/2, -a/2], F=[col, col^2, 1]).

Precision: both factors are split into bf16 hi+lo parts and stacked to K=51
(lhsT rows [Fhi;Flo;Fhi] x rhs rows [Whi;Whi;Wlo]) which reproduces the fp32
product to ~2^-18 relative while streaming the PE at full bf16 rate.

Sharding: hills are split across the 8 NeuronCores (2048 each); every core
computes a partial [4096] potential and the host sums the partials.

Per-core dataflow (ACT-engine bound, ~1 exp/lane/cycle):
  for each of 32 point-tiles (128 points):
    4x matmul [K=51, M=128, N=512] -> one [128, 2048] PSUM tile (4 banks)
    1x ACTIVATE(Exp) in-place over the PSUM tile with accum_out -> [128,1]
       (fused exp + sum over the 2048 local hills)
  PSUM pool ping-pongs banks 0-3/4-7 so the PE fills one half while the ACT
  engine drains the other. Partials collect into one [128, 32] tile, written
  out in two batched DMAs.
"""

import numpy as np
import ml_dtypes

import concourse.bacc as bacc
import concourse.mybir as mybir
import concourse.tile as tile
from concourse import bass_utils

H, P, D = 16384, 4096, 8
NCORES = 8
HL = H // NCORES          # hills per core
K = 51                    # 3 x 17 stacked hi/lo blocks
PT = 128                  # points per tile (PSUM partitions)
NPT = P // PT             # 32 p-tiles
HC = 512                  # hills per matmul (one PSUM bank of f32)
NHC = HL // HC            # 4 matmuls per p-tile

BF16 = mybir.dt.bfloat16
F32 = mybir.dt.float32

_NC_CACHE = None


def _build_nc():
    nc = bacc.Bacc(
        "TRN2",
        target_bir_lowering=False,
        debug=False,
        enable_asserts=False,
        num_devices=NCORES,
    )
    ft = nc.dram_tensor("ft", [K, P], BF16, kind="ExternalInput").ap()
    wt = nc.dram_tensor("wt", [K, HL], BF16, kind="ExternalInput").ap()
    # out[p_lane, n_tile]: row-major so the final DMA writes 128B runs per
    # partition. Host does out.T.ravel() to get the [4096] point order.
    out = nc.dram_tensor("out", [PT, NPT], F32, kind="ExternalOutput").ap()

    with tile.TileContext(nc) as tc:
        with (
            tc.tile_pool(name="const", bufs=1) as cpool,
            tc.tile_pool(name="psum", bufs=2, space="PSUM") as ppool,
        ):
            ftt = cpool.tile([K, P], BF16)
            wtt = cpool.tile([K, HL], BF16)
            acc = cpool.tile([PT, NPT], F32)

            # Critical-path loads: wt gates p-tile 0's matmuls — two
            # partition-halves on the sync ring engage two DMA-engine sets.
            # ft streams on the scalar ring in staged chunks (p-tile i only
            # needs ft[:, 128i:128(i+1)], so later chunks can land late).
            # wt's first hill-chunk goes alone so MM(0,0) starts early; the
            # remaining columns follow as partition-halves on both rings.
            nc.sync.dma_start(wtt[:, 0:HC], wt[:, 0:HC])
            nc.sync.dma_start(wtt[0:26, HC:HL], wt[0:26, HC:HL])
            nc.scalar.dma_start(ftt[:, 0:PT], ft[:, 0:PT])
            nc.scalar.dma_start(wtt[26:K, HC:HL], wt[26:K, HC:HL])
            nc.scalar.dma_start(ftt[:, PT:1024], ft[:, PT:1024])
            nc.scalar.dma_start(ftt[:, 1024:2432], ft[:, 1024:2432])
            nc.scalar.dma_start(ftt[:, 2432:P], ft[:, 2432:P])

            for i in range(NPT):
                pt = ppool.tile([PT, HL], F32)  # 4 PSUM banks
                for j in range(NHC):
                    nc.tensor.matmul(
                        pt[:, j * HC : (j + 1) * HC],
                        lhsT=ftt[:, i * PT : (i + 1) * PT],
                        rhs=wtt[:, j * HC : (j + 1) * HC],
                        start=True,
                        stop=True,
                    )
                nc.scalar.activation(
                    pt[:],
                    pt[:],
                    mybir.ActivationFunctionType.Exp,
                    scale=1.0,
                    accum_out=acc[:, i : i + 1],
                )
                if i == NPT // 2 - 1:
                    nc.sync.dma_start(out[:, : NPT // 2], acc[:, : NPT // 2])
            nc.sync.dma_start(out[:, NPT // 2 :], acc[:, NPT // 2 :])

    nc.compile()
    return nc


def _get_nc():
    global _NC_CACHE
    if _NC_CACHE is None:
        _NC_CACHE = _build_nc()
    return _NC_CACHE


def _split_bf16(x64):
    hi = x64.astype(ml_dtypes.bfloat16)
    lo = (x64 - hi.astype(np.float64)).astype(ml_dtypes.bfloat16)
    return hi, lo


def _prepare_inputs(col, cen, wdt, hgt):
    col64 = col.astype(np.float64)
    cen64 = cen.astype(np.float64)
    wdt64 = wdt.astype(np.float64)
    hgt64 = np.maximum(hgt.astype(np.float64), 1e-38)

    c = 1.0 / (wdt64 * wdt64)                                     # [H, D]
    a = np.sum(cen64 * cen64 * c, axis=1) - 2.0 * np.log(hgt64)   # [H]
    W = np.concatenate([cen64 * c, -0.5 * c, -0.5 * a[:, None]], axis=1)  # [H, 17]
    F = np.concatenate([col64, col64 * col64, np.ones((P, 1))], axis=1)   # [P, 17]

    Whi, Wlo = _split_bf16(W)
    Fhi, Flo = _split_bf16(F)

    ft = np.ascontiguousarray(np.concatenate([Fhi.T, Flo.T, Fhi.T], axis=0))  # [51, P]
    wt_full = np.concatenate([Whi.T, Whi.T, Wlo.T], axis=0)                   # [51, H]
    wts = [
        np.ascontiguousarray(wt_full[:, i * HL : (i + 1) * HL]) for i in range(NCORES)
    ]
    return ft, wts


def run_on_hw(col, cen, wdt, hgt, trace=False):
    """Run the SPMD kernel on 8 cores; returns (out[P] f32, BassKernelResults)."""
    ft, wts = _prepare_inputs(col, cen, wdt, hgt)
    nc = _get_nc()
    in_maps = [{"ft": ft, "wt": wts[i]} for i in range(NCORES)]
    res = bass_utils.run_bass_kernel_spmd(
        nc, in_maps, core_ids=list(range(NCORES)), trace=trace
    )
    total = np.zeros(P, dtype=np.float64)
    for r in res.results:
        total += r["out"].T.reshape(P).astype(np.float64)
    return total.astype(np.float32), res


def kernel(col, cen, wdt, hgt):
    out, _ = run_on_hw(col, cen, wdt, hgt, trace=False)
    return out

